# revision 22
# baseline (speedup 1.0000x reference)
"""Trainium2 Bass kernel for topk_masking:  out = X + alpha * (top32_mask(A) @ X).

Row-parallel across 8 NeuronCores (A sharded [1024, 8192] per core, X
replicated).  A is shipped as fp16 of (A - SHIFT), SHIFT ~ the expected
32nd-largest value per row: rounding is monotone, so the fp16 top-32
equals the fp32 top-32 unless two boundary values collide on the fp16
grid -- exactly what the count detector flags for host recomputation.
This halves the dominant A DMA traffic.

Work is organized in PAIRS of 128-row batches (256 rows) per core, with
every cross-stage handoff placed so no engine's in-order queue ever
waits on a later pipeline stage:
  * DMA (sync ring): two 2 MB fp16 loads of the pair's A rows.
  * VectorE per batch: per-512-segment max8, then 4 rounds of
    max+match_replace -> top-32; t32 = 32nd largest (segment overflow
    detected and host-fixed).
  * ScalarE per batch: maskpm = Sign(A + (2^-25 - t32)) in fp8e4, fused
    accum_out -> per-row count detector.
  * GPSIMD dma_gather(transpose): ONE 256-row xbar transpose per pair
    (16-bit tokens hold column pairs).
  * TensorE with SWAPPED operands: stationary = contiguous bf16 X chunks
    (FWL hides the 128 weight loads), moving = transposed fp8 mask via
    stride-2 APs, output = transposed [D, rows] block.  X is pre-scaled
    by alpha/2 host-side.  The residual X_self and the +-1-form colsum
    correction enter as 8 extra mask-independent matmuls per pair
    (interleaved-identity + row-0 colsum weights in bf16 hi/lo), issued
    FIRST in the accumulation group, so no vector-engine epilogue math
    is needed at all.
  * ScalarE (deferred two pairs): plain Copy psum -> bf16, store via
    sync ring.  Host un-transposes the [D, 1024] per-core result.
Host: rows whose detector count != 2K - N are recomputed exactly.
"""

import os
import numpy as np

N = 8192
D = 256
K = 32
NCORES = 8
RPC = N // NCORES          # rows per core = 1024
BATCH = 128
NBATCH = RPC // BATCH      # 8
NPAIR = NBATCH // 2        # 4
SEG = int(os.environ.get("TOPK_SEG", "512"))
NCH2 = N // 256            # 32 token-chunks (column pairs x 128)
NEG_BIG = -60000.0         # fp16-representable sentinel
SHIFT = 2.66               # ~E[32nd largest of 8192 N(0,1)]
EPS_TIE = float(2.0 ** -25)  # keeps Sign(y - t32) > 0 for y == t32

last_results = None
_nc_cache = {}


def _build_cached(loop_reps=1, seg=None):
    key = (loop_reps, seg or SEG)
    if key not in _nc_cache:
        _nc_cache[key] = _build(loop_reps, seg)
    return _nc_cache[key]


def _build(loop_reps=1, seg=None):
    import concourse.bacc as bacc
    import concourse.mybir as mybir
    from concourse.tile import TileContext
    from concourse import library_config

    seg = seg or SEG
    nseg = N // seg            # segments per full row (over original columns)
    seg2 = seg // 2            # prefiltered elements per segment
    half = N // 2
    fp32 = mybir.dt.float32
    fp16 = mybir.dt.float16
    bf16 = mybir.dt.bfloat16
    fp8 = mybir.dt.float8e4
    u16 = mybir.dt.uint16
    mx = mybir.AluOpType.max
    add = mybir.AluOpType.add
    Sign = mybir.ActivationFunctionType.Sign
    Copy = mybir.ActivationFunctionType.Copy

    nc = bacc.Bacc("TRN2", debug=False)
    a_in = nc.declare_dram_parameter("a", [RPC, N], fp16, isOutput=False)
    xb_in = nc.declare_dram_parameter("xb", [128, NCH2 * 2 * D], bf16, isOutput=False)
    xs_in = nc.declare_dram_parameter("xsb", [128, NPAIR * 2 * D], bf16, isOutput=False)
    cs_in = nc.declare_dram_parameter("csw", [128, 4 * 128], bf16, isOutput=False)
    i2_in = nc.declare_dram_parameter("i2", [128, 3 * 256], fp8, isOutput=False)
    ti_in = nc.declare_dram_parameter("tidx", [128, 16], mybir.dt.int16, isOutput=False)
    out_ext = nc.declare_dram_parameter("out", [D, RPC], bf16, isOutput=True)
    cnt_ext = nc.declare_dram_parameter("count", [RPC, 1], fp32, isOutput=True)

    abufs = int(os.environ.get("TOPK_ABUFS", "2"))

    with TileContext(nc) as tc:
        with (
            tc.tile_pool(name="persist", bufs=1) as persist,
            tc.tile_pool(name="apool", bufs=abufs) as apool,
            tc.tile_pool(name="mpool", bufs=int(os.environ.get("TOPK_MBUFS", "2"))) as mpool,
            tc.tile_pool(name="mtpool", bufs=int(os.environ.get("TOPK_MTBUFS", "2"))) as mtpool,
            tc.tile_pool(name="small", bufs=3) as small,
            tc.tile_pool(name="psum", bufs=3, space="PSUM") as psum_pool,
        ):
            nc.gpsimd.load_library(library_config.mlp)

            tidx = persist.tile([128, 16], mybir.dt.int16)
            nc.scalar.dma_start(out=tidx[:], in_=ti_in[:])

            # Xs resident in bf16, even/odd token-chunk layout, pre-scaled by
            # alpha/2 on host: xb[p, c, e*D + d] = (alpha/2) X[c*256+2p+e, d]
            xb = persist.tile([128, NCH2 * 2 * D], bf16)
            nc.scalar.dma_start(out=xb[:], in_=xb_in[:])
            xv = xb[:].rearrange("p (c d) -> p c d", d=2 * D)

            # unscaled bf16 X rows of this core: xsb[p, pair, e*D + d]
            xsb = persist.tile([128, NPAIR * 2 * D], bf16)
            nc.scalar.dma_start(out=xsb[:], in_=xs_in[:])
            xsv = xsb[:].rearrange("p (q d) -> p q d", d=2 * D)

            # csw[k, (lvl*2+dh)*128 + m] = k==0 ? cs_lvl[dh*128+m] : 0
            csw = persist.tile([128, 4 * 128], bf16)
            nc.scalar.dma_start(out=csw[:], in_=cs_in[:])
            # i2[k, e*256 + n] = (n == 2k+e); i2[k, 512+n] = (k == 0)
            i2 = persist.tile([128, 3 * 256], fp8)
            nc.scalar.dma_start(out=i2[:], in_=i2_in[:])

            cnt_all = persist.tile([128, NBATCH], fp32)

            at_tiles = {}

            def load_pair(p):
                # [128, 2*N]: batch 2p rows in first N cols, batch 2p+1 next
                atile = apool.tile([128, 2 * N], fp16, tag="at")
                for r2 in range(2):
                    nc.sync.dma_start(
                        out=atile[:, r2 * N:(r2 + 1) * N],
                        in_=a_in[(2 * p + r2) * BATCH:(2 * p + r2 + 1) * BATCH, :])
                at_tiles[p] = atile

            if loop_reps == 1:
                load_pair(0)
                load_pair(1)

            pending = []

            def flush_pending(keep=0):
                if len(pending) <= keep:
                    return
                ps0, ps1, pp = pending.pop(0)
                ot = small.tile([128, 2 * 256], bf16, tag="ot")
                for dh, ps in ((0, ps0), (1, ps1)):
                    nc.scalar.activation(out=ot[:, dh * 256:(dh + 1) * 256],
                                         in_=ps[:], func=Copy)
                nc.sync.dma_start(
                    out=out_ext.rearrange("(h d) r -> d h r", d=128)[
                        :, :, pp * 256:(pp + 1) * 256],
                    in_=ot[:].rearrange("d (h r) -> d h r", r=256))

            def pair_body(p):  # noqa: C901
                if p + 2 < NPAIR:
                    load_pair(p + 2)
                atile = at_tiles.pop(p)
                maskb = mpool.tile([128, 2 * N], fp8, tag="mb")
                maskT = mtpool.tile([128, NCH2 * 256], u16, tag="mt")

                # mask-independent matmuls first: psum <- colsum(hi+lo) +
                # X_self rows of this pair (via interleaved identities).
                # These need no gather, warm the PE early, and replace any
                # vector-engine epilogue math.
                ps0 = psum_pool.tile([128, 256], fp32, tag="ps0")
                ps1 = psum_pool.tile([128, 256], fp32, tag="ps1")
                pst = (ps0, ps1)
                for dh in range(2):
                    for lvl in range(2):
                        nc.tensor.matmul(
                            pst[dh][:],
                            lhsT=csw[:, (lvl * 2 + dh) * 128:(lvl * 2 + dh + 1) * 128],
                            rhs=i2[:, 512:768],
                            start=(lvl == 0), stop=False)
                    for e in range(2):
                        nc.tensor.matmul(
                            pst[dh][:],
                            lhsT=xsv[:, p, e * D + dh * 128:e * D + (dh + 1) * 128],
                            rhs=i2[:, e * 256:(e + 1) * 256],
                            start=False, stop=False)

                for r2 in range(2):
                    b = 2 * p + r2
                    av = atile[:, r2 * N:(r2 + 1) * N]

                    cands = small.tile([128, nseg * 8], fp16, tag="cd")
                    for s in range(nseg):
                        nc.vector.max(out=cands[:, s * 8:(s + 1) * 8],
                                      in_=av[:, s * seg:(s + 1) * seg])
                    v8 = small.tile([128, K], fp16, tag="v8")
                    for r in range(4):
                        nc.vector.max(out=v8[:, r * 8:(r + 1) * 8], in_=cands[:])
                        if r < 3:
                            nc.vector.match_replace(
                                out=cands[:],
                                in_to_replace=v8[:, r * 8:(r + 1) * 8],
                                in_values=cands[:], imm_value=NEG_BIG)

                    # ntp = 2^-25 - t32 (fp32); Sign(y + ntp): +1 iff y >= t32
                    ntp = small.tile([128, 1], fp32, tag="ntp")
                    nc.scalar.activation(out=ntp[:], in_=v8[:, K - 1:K],
                                         func=Copy, scale=-1.0, bias=EPS_TIE)
                    nc.scalar.activation(
                        out=maskb[:, r2 * N:(r2 + 1) * N], in_=av, func=Sign,
                        bias=ntp[:, 0:1], scale=1.0,
                        accum_out=cnt_all[:, b:b + 1])

                    if r2 == 0:
                        # epilogue of the pair TWO iterations back: its
                        # matmuls are long done -> no queue ever blocks
                        flush_pending(keep=1)

                gather_pair(p, maskb, maskT)
                pair_matmuls(p, maskT, pst)
                pending.append((ps0, ps1, p))

            def gather_pair(p, maskb, maskT):
                # one 256-row xbar transpose per pair (SWDGE on Pool)
                nc.gpsimd.dma_gather(
                    out_ap=maskT[:].rearrange("p (c i) -> p c i", i=256),
                    in_ap=maskb[:], idxs_ap=tidx[:],
                    num_idxs=256, num_idxs_reg=256, elem_size=N // 2,
                    transpose=True,
                    sbuf_tokens_per_rank=128, sbuf_free_dim_per_rank=N)

            def pair_matmuls(p, maskT, pst):
                # mview[p, c, i, e] = mask[row i of pair, col 256c + 2p + e]
                mview = maskT[:].bitcast(fp8).rearrange(
                    "p (c i e) -> p c i e", i=256, e=2)
                for c in range(NCH2):
                    for e in range(2):
                        for dh in range(2):
                            nc.tensor.matmul(
                                pst[dh][:],
                                lhsT=xv[:, c, e * D + dh * 128:
                                        e * D + (dh + 1) * 128],
                                rhs=mview[:, c, :, e],
                                start=False,
                                stop=(c == NCH2 - 1 and e == 1))

            if loop_reps == 1:
                for p in range(NPAIR):
                    pair_body(p)
                while pending:
                    flush_pending()
            elif bool(int(os.environ.get("TOPK_SIM_UNROLL", "0"))):
                # python-unrolled reps: same instruction stream the For_i
                # body would execute, but simulatable by TimelineSim
                for _ in range(loop_reps):
                    load_pair(0)
                    load_pair(1)
                    for p in range(NPAIR):
                        pair_body(p)
                    while pending:
                        flush_pending()
            else:
                with tc.For_i(0, loop_reps, 1):
                    load_pair(0)
                    load_pair(1)
                    for p in range(NPAIR):
                        pair_body(p)
                    while pending:
                        flush_pending()

            # counts: cnt_all[p, b] -> count[b*128 + p]
            nc.sync.dma_start(
                out=cnt_ext.rearrange("(b p) one -> p (b one)", p=128),
                in_=cnt_all[:],
            )
    nc.compile()
    return nc


def _tidx():
    t = np.zeros((16, 16), np.int16)
    for i in range(256):
        t[i % 16, i // 16] = i
    return np.tile(t, (8, 1))


def make_in_maps(A, X, alpha):
    import ml_dtypes
    bf = ml_dtypes.bfloat16
    f8 = ml_dtypes.float8_e4m3
    half_a = np.float32(alpha) / np.float32(2.0)
    Xs = (X * half_a).astype(bf)
    # xb layout: xb[p, c, e*D + d] = (alpha/2) X[c*256 + 2p + e, d]
    X2 = Xs.reshape(NCH2, 128, 2, D)
    xb = np.ascontiguousarray(np.transpose(X2, (1, 0, 2, 3))).reshape(
        128, NCH2 * 2 * D)
    # colsum of the bf16-rounded scaled X, split bf16 hi/lo
    cs = Xs.astype(np.float64).sum(axis=0).astype(np.float32)
    cs_hi = cs.astype(bf)
    cs_lo = (cs - cs_hi.astype(np.float32)).astype(bf)
    csw = np.zeros((128, 4 * 128), bf)
    for lvl, csl in ((0, cs_hi), (1, cs_lo)):
        for dh in range(2):
            csw[0, (lvl * 2 + dh) * 128:(lvl * 2 + dh + 1) * 128] = \
                csl[dh * 128:(dh + 1) * 128]
    # i2: interleaved identities + ones row
    i2 = np.zeros((128, 3 * 256), f8)
    for kk in range(128):
        i2[kk, 2 * kk] = 1.0
        i2[kk, 256 + 2 * kk + 1] = 1.0
    i2[0, 512:768] = 1.0
    tidx = _tidx()
    # fp16 of (A - SHIFT): monotone; grid collisions at the boundary detected
    a16 = np.clip(A - np.float32(SHIFT), -60000.0, 60000.0).astype(np.float16)
    maps = []
    for c in range(NCORES):
        Xc = X[c * RPC:(c + 1) * RPC]          # this core's own rows
        xs4 = Xc.astype(bf).reshape(NPAIR, 128, 2, D)
        xsb = np.ascontiguousarray(np.transpose(xs4, (1, 0, 2, 3))).reshape(
            128, NPAIR * 2 * D)
        maps.append({
            "a": a16[c * RPC:(c + 1) * RPC],
            "xb": xb,
            "xsb": xsb,
            "csw": csw,
            "i2": i2,
            "tidx": tidx,
        })
    return maps


def kernel(**inputs):
    global last_results
    from concourse.bass_utils import run_bass_kernel_spmd

    A = np.ascontiguousarray(np.asarray(inputs["A"], dtype=np.float32))
    X = np.ascontiguousarray(np.asarray(inputs["X"], dtype=np.float32))
    alpha = np.float32(np.asarray(inputs["alpha"]))
    k = int(np.asarray(inputs["k"]))
    assert A.shape == (N, N) and X.shape == (N, D)
    if k != K or alpha == 0.0:
        # Safety net for an unexpected k (or alpha=0): exact host computation.
        idx = np.argsort(-A, axis=1, kind="stable")[:, :k]
        agg = X[idx].sum(axis=1, dtype=np.float32)
        return (X + alpha * agg).astype(np.float32)

    nc = _build_cached()
    in_maps = make_in_maps(A, X, alpha)

    trace = bool(int(os.environ.get("TOPK_TRACE", "0")))
    res = run_bass_kernel_spmd(nc, in_maps, core_ids=list(range(NCORES)),
                               trace=trace)
    last_results = res

    # per-core output is [D, RPC] bf16; un-transpose and stack rows
    out = np.concatenate(
        [r["out"].astype(np.float32).T for r in res.results], axis=0)
    accs = np.concatenate([r["count"] for r in res.results], axis=0)[:, 0]

    # Host fallback for rows where the device selection is not exactly top-k
    # (fp16 boundary ties, prefilter mate collisions, segment overflow,
    # Sign hitting exact zero).
    bad = np.flatnonzero(accs != np.float32(2 * K - N))
    for r in bad:
        order = np.argsort(-A[r], kind="stable")[:K]
        out[r] = X[r] + alpha * X[order].sum(axis=0, dtype=np.float32)

    return np.ascontiguousarray(out, dtype=np.float32)


# revision 25
# speedup vs baseline: 1.6303x; 1.6303x over previous
"""Trainium2 Bass kernel for topk_masking:  out = X + alpha * (top32_mask(A) @ X).

Row-parallel across 8 NeuronCores (A sharded [1024, 8192] per core, X
replicated).  A is shipped as fp16 of (A - SHIFT), SHIFT ~ the expected
32nd-largest value per row: rounding is monotone, so the fp16 top-32
equals the fp32 top-32 unless two boundary values collide on the fp16
grid -- exactly what the count detector flags for host recomputation.
This halves the dominant A DMA traffic.

Work is organized in PAIRS of 128-row batches (256 rows) per core, with
every cross-stage handoff placed so no engine's in-order queue ever
waits on a later pipeline stage:
  * DMA (sync ring): two 2 MB fp16 loads of the pair's A rows.
  * VectorE per batch: per-512-segment max8, then 4 rounds of
    max+match_replace -> top-32; t32 = 32nd largest (segment overflow
    detected and host-fixed).
  * ScalarE per batch: maskpm = Sign(A + (2^-25 - t32)) in fp8e4, fused
    accum_out -> per-row count detector.
  * GPSIMD dma_gather(transpose): ONE 256-row xbar transpose per pair
    (16-bit tokens hold column pairs).
  * TensorE with SWAPPED operands: stationary = contiguous bf16 X chunks
    (FWL hides the 128 weight loads), moving = transposed fp8 mask via
    stride-2 APs, output = transposed [D, rows] block.  X is pre-scaled
    by alpha/2 host-side.  The residual X_self and the +-1-form colsum
    correction enter as 8 extra mask-independent matmuls per pair
    (interleaved-identity + row-0 colsum weights in bf16 hi/lo), issued
    FIRST in the accumulation group, so no vector-engine epilogue math
    is needed at all.
  * ScalarE (deferred two pairs): plain Copy psum -> bf16, store via
    sync ring.  Host un-transposes the [D, 1024] per-core result.
Host: rows whose detector count != 2K - N are recomputed exactly.
"""

import os
import numpy as np

N = 8192
D = 256
K = 32
NCORES = 8
RPC = N // NCORES          # rows per core = 1024
BATCH = 128
NBATCH = RPC // BATCH      # 8
NPAIR = NBATCH // 2        # 4
SEG = int(os.environ.get("TOPK_SEG", "512"))
NCH2 = N // 256            # 32 token-chunks (column pairs x 128)
NEG_BIG = -60000.0         # fp16-representable sentinel
SHIFT = 2.66               # ~E[32nd largest of 8192 N(0,1)]
EPS_TIE = float(2.0 ** -25)  # keeps Sign(y - t32) > 0 for y == t32

last_results = None
_nc_cache = {}


def _build_cached(loop_reps=1, seg=None):
    key = (loop_reps, seg or SEG)
    if key not in _nc_cache:
        _nc_cache[key] = _build(loop_reps, seg)
    return _nc_cache[key]


def _build(loop_reps=1, seg=None):
    import concourse.bacc as bacc
    import concourse.mybir as mybir
    from concourse.tile import TileContext
    from concourse import library_config

    seg = seg or SEG
    nseg = N // seg            # segments per full row (over original columns)
    seg2 = seg // 2            # prefiltered elements per segment
    half = N // 2
    fp32 = mybir.dt.float32
    fp16 = mybir.dt.float16
    bf16 = mybir.dt.bfloat16
    fp8 = mybir.dt.float8e4
    u16 = mybir.dt.uint16
    mx = mybir.AluOpType.max
    add = mybir.AluOpType.add
    Sign = mybir.ActivationFunctionType.Sign
    Copy = mybir.ActivationFunctionType.Copy

    nc = bacc.Bacc("TRN2", debug=False)
    a_in = nc.declare_dram_parameter("a", [RPC, N], fp16, isOutput=False)
    xb_in = nc.declare_dram_parameter("xb", [128, NCH2 * 2 * D], bf16, isOutput=False)
    xs_in = nc.declare_dram_parameter("xsb", [128, NPAIR * 2 * D], bf16, isOutput=False)
    cs_in = nc.declare_dram_parameter("csw", [128, 4 * 128], bf16, isOutput=False)
    i2_in = nc.declare_dram_parameter("i2", [128, 3 * 256], fp8, isOutput=False)
    ti_in = nc.declare_dram_parameter("tidx", [128, 16], mybir.dt.int16, isOutput=False)
    out_ext = nc.declare_dram_parameter("out", [D, RPC], bf16, isOutput=True)
    cnt_ext = nc.declare_dram_parameter("count", [RPC, 1], fp32, isOutput=True)

    abufs = int(os.environ.get("TOPK_ABUFS", "2"))

    rolled = loop_reps != 1
    with TileContext(nc) as tc:
        with (
            tc.tile_pool(name="persist", bufs=1) as persist,
            tc.tile_pool(name="apool", bufs=abufs) as apool,
            tc.tile_pool(name="mpool", bufs=int(os.environ.get("TOPK_MBUFS", "2"))) as mpool,
            tc.tile_pool(name="mtpool", bufs=int(os.environ.get("TOPK_MTBUFS", "2"))) as mtpool,
            tc.tile_pool(name="small", bufs=3) as small,
            tc.tile_pool(name="psum", bufs=1, space="PSUM") as psum_pool,
        ):
            nc.gpsimd.load_library(library_config.mlp)

            tidx = persist.tile([128, 16], mybir.dt.int16)
            nc.scalar.dma_start(out=tidx[:], in_=ti_in[:])

            # Xs resident in bf16, even/odd token-chunk layout, pre-scaled by
            # alpha/2 on host: xb[p, c, e*D + d] = (alpha/2) X[c*256+2p+e, d]
            xb = persist.tile([128, NCH2 * 2 * D], bf16)
            nc.scalar.dma_start(out=xb[:], in_=xb_in[:])
            xv = xb[:].rearrange("p (c d) -> p c d", d=2 * D)

            # unscaled bf16 X rows of this core: xsb[p, pair, e*D + d]
            xsb = persist.tile([128, NPAIR * 2 * D], bf16)
            nc.scalar.dma_start(out=xsb[:], in_=xs_in[:])
            xsv = xsb[:].rearrange("p (q d) -> p q d", d=2 * D)

            # csw[k, (lvl*2+dh)*128 + m] = k==0 ? cs_lvl[dh*128+m] : 0
            csw = persist.tile([128, 4 * 128], bf16)
            nc.scalar.dma_start(out=csw[:], in_=cs_in[:])
            # i2[k, e*256 + n] = (n == 2k+e); i2[k, 512+n] = (k == 0)
            i2 = persist.tile([128, 3 * 256], fp8)
            nc.scalar.dma_start(out=i2[:], in_=i2_in[:])

            cnt_all = persist.tile([128, NBATCH], fp32)

            # one persistent psum accumulator per (pair slot, d-half): lets
            # the epilogue of slot P roll into the NEXT loop iteration's
            # body without any pool rotation barrier
            pslots = []
            for q in range(NPAIR):
                ps_a = psum_pool.tile([128, 256], fp32, tag=f"ps{q}0")
                ps_b = psum_pool.tile([128, 256], fp32, tag=f"ps{q}1")
                pslots.append((ps_a, ps_b))

            at_tiles = {}

            def load_pair(p):
                # [128, 2*N]: batch 2p rows in first N cols, batch 2p+1 next
                atile = apool.tile([128, 2 * N], fp16, tag="at")
                for r2 in range(2):
                    nc.sync.dma_start(
                        out=atile[:, r2 * N:(r2 + 1) * N],
                        in_=a_in[(2 * p + r2) * BATCH:(2 * p + r2 + 1) * BATCH, :])
                at_tiles[p] = atile

            if loop_reps == 1:
                load_pair(0)
                load_pair(1)

            def flush_slot(pp):
                ps0, ps1 = pslots[pp]
                ot = small.tile([128, 2 * 256], bf16, tag="ot")
                for dh, ps in ((0, ps0), (1, ps1)):
                    nc.scalar.activation(out=ot[:, dh * 256:(dh + 1) * 256],
                                         in_=ps[:], func=Copy)
                nc.sync.dma_start(
                    out=out_ext.rearrange("(h d) r -> d h r", d=128)[
                        :, :, pp * 256:(pp + 1) * 256],
                    in_=ot[:].rearrange("d (h r) -> d h r", r=256))

            def pair_body(p):  # noqa: C901
                if p + 2 < NPAIR:
                    load_pair(p + 2)
                atile = at_tiles.pop(p)
                maskb = mpool.tile([128, 2 * N], fp8, tag="mb")
                maskT = mtpool.tile([128, NCH2 * 256], u16, tag="mt")

                # mask-independent matmuls first: psum <- colsum(hi+lo) +
                # X_self rows of this pair (via interleaved identities).
                # These need no gather, warm the PE early, and replace any
                # vector-engine epilogue math.
                pst = pslots[p]
                for dh in range(2):
                    for lvl in range(2):
                        nc.tensor.matmul(
                            pst[dh][:],
                            lhsT=csw[:, (lvl * 2 + dh) * 128:(lvl * 2 + dh + 1) * 128],
                            rhs=i2[:, 512:768],
                            start=(lvl == 0), stop=False)
                    for e in range(2):
                        nc.tensor.matmul(
                            pst[dh][:],
                            lhsT=xsv[:, p, e * D + dh * 128:e * D + (dh + 1) * 128],
                            rhs=i2[:, e * 256:(e + 1) * 256],
                            start=False, stop=False)

                for r2 in range(2):
                    b = 2 * p + r2
                    av = atile[:, r2 * N:(r2 + 1) * N]

                    cands = small.tile([128, nseg * 8], fp16, tag="cd")
                    for s in range(nseg):
                        nc.vector.max(out=cands[:, s * 8:(s + 1) * 8],
                                      in_=av[:, s * seg:(s + 1) * seg])
                    v8 = small.tile([128, K], fp16, tag="v8")
                    for r in range(4):
                        nc.vector.max(out=v8[:, r * 8:(r + 1) * 8], in_=cands[:])
                        if r < 3:
                            nc.vector.match_replace(
                                out=cands[:],
                                in_to_replace=v8[:, r * 8:(r + 1) * 8],
                                in_values=cands[:], imm_value=NEG_BIG)

                    # ntp = 2^-25 - t32 (fp32); Sign(y + ntp): +1 iff y >= t32
                    ntp = small.tile([128, 1], fp32, tag="ntp")
                    nc.scalar.activation(out=ntp[:], in_=v8[:, K - 1:K],
                                         func=Copy, scale=-1.0, bias=EPS_TIE)
                    nc.scalar.activation(
                        out=maskb[:, r2 * N:(r2 + 1) * N], in_=av, func=Sign,
                        bias=ntp[:, 0:1], scale=1.0,
                        accum_out=cnt_all[:, b:b + 1])

                    if r2 == 0 and (rolled or p >= 2):
                        # epilogue of the slot TWO pairs back (previous
                        # loop iteration for p < 2): its matmuls are long
                        # done -> no engine queue ever blocks, and the
                        # pipeline rolls across For_i iterations
                        flush_slot((p + 2) % NPAIR)

                gather_pair(p, maskb, maskT)
                pair_matmuls(p, maskT, pst)

            def gather_pair(p, maskb, maskT):
                # one 256-row xbar transpose per pair (SWDGE on Pool)
                nc.gpsimd.dma_gather(
                    out_ap=maskT[:].rearrange("p (c i) -> p c i", i=256),
                    in_ap=maskb[:], idxs_ap=tidx[:],
                    num_idxs=256, num_idxs_reg=256, elem_size=N // 2,
                    transpose=True,
                    sbuf_tokens_per_rank=128, sbuf_free_dim_per_rank=N)

            def pair_matmuls(p, maskT, pst):
                # mview[p, c, i, e] = mask[row i of pair, col 256c + 2p + e]
                mview = maskT[:].bitcast(fp8).rearrange(
                    "p (c i e) -> p c i e", i=256, e=2)
                for c in range(NCH2):
                    for e in range(2):
                        for dh in range(2):
                            nc.tensor.matmul(
                                pst[dh][:],
                                lhsT=xv[:, c, e * D + dh * 128:
                                        e * D + (dh + 1) * 128],
                                rhs=mview[:, c, :, e],
                                start=False,
                                stop=(c == NCH2 - 1 and e == 1))

            def warm_slots():
                # the rolled epilogue reads slots 2,3 before the first
                # in-loop write: initialize every slot so the trace never
                # reads an unwritten psum (values are overwritten/flushed)
                for q in range(NPAIR):
                    for dh in range(2):
                        nc.tensor.matmul(
                            pslots[q][dh][:], lhsT=csw[:, 0:128],
                            rhs=i2[:, 512:768], start=True, stop=True)

            if loop_reps == 1:
                warm_slots()
                for p in range(NPAIR):
                    pair_body(p)
                flush_slot(2)
                flush_slot(3)
            elif bool(int(os.environ.get("TOPK_SIM_UNROLL", "0"))):
                # python-unrolled reps: same instruction stream the For_i
                # body would execute, but simulatable by TimelineSim
                warm_slots()
                for _ in range(loop_reps):
                    load_pair(0)
                    load_pair(1)
                    for p in range(NPAIR):
                        pair_body(p)
                flush_slot(2)
                flush_slot(3)
            else:
                warm_slots()
                with tc.For_i(0, loop_reps, 1):
                    load_pair(0)
                    load_pair(1)
                    for p in range(NPAIR):
                        pair_body(p)
                flush_slot(2)
                flush_slot(3)

            # counts: cnt_all[p, b] -> count[b*128 + p]
            nc.sync.dma_start(
                out=cnt_ext.rearrange("(b p) one -> p (b one)", p=128),
                in_=cnt_all[:],
            )
    nc.compile()
    return nc


def _tidx():
    t = np.zeros((16, 16), np.int16)
    for i in range(256):
        t[i % 16, i // 16] = i
    return np.tile(t, (8, 1))


def make_in_maps(A, X, alpha):
    import ml_dtypes
    bf = ml_dtypes.bfloat16
    f8 = ml_dtypes.float8_e4m3
    half_a = np.float32(alpha) / np.float32(2.0)
    Xs = (X * half_a).astype(bf)
    # xb layout: xb[p, c, e*D + d] = (alpha/2) X[c*256 + 2p + e, d]
    X2 = Xs.reshape(NCH2, 128, 2, D)
    xb = np.ascontiguousarray(np.transpose(X2, (1, 0, 2, 3))).reshape(
        128, NCH2 * 2 * D)
    # colsum of the bf16-rounded scaled X, split bf16 hi/lo
    cs = Xs.astype(np.float64).sum(axis=0).astype(np.float32)
    cs_hi = cs.astype(bf)
    cs_lo = (cs - cs_hi.astype(np.float32)).astype(bf)
    csw = np.zeros((128, 4 * 128), bf)
    for lvl, csl in ((0, cs_hi), (1, cs_lo)):
        for dh in range(2):
            csw[0, (lvl * 2 + dh) * 128:(lvl * 2 + dh + 1) * 128] = \
                csl[dh * 128:(dh + 1) * 128]
    # i2: interleaved identities + ones row
    i2 = np.zeros((128, 3 * 256), f8)
    for kk in range(128):
        i2[kk, 2 * kk] = 1.0
        i2[kk, 256 + 2 * kk + 1] = 1.0
    i2[0, 512:768] = 1.0
    tidx = _tidx()
    # fp16 of (A - SHIFT): monotone; grid collisions at the boundary detected
    a16 = np.clip(A - np.float32(SHIFT), -60000.0, 60000.0).astype(np.float16)
    maps = []
    for c in range(NCORES):
        Xc = X[c * RPC:(c + 1) * RPC]          # this core's own rows
        xs4 = Xc.astype(bf).reshape(NPAIR, 128, 2, D)
        xsb = np.ascontiguousarray(np.transpose(xs4, (1, 0, 2, 3))).reshape(
            128, NPAIR * 2 * D)
        maps.append({
            "a": a16[c * RPC:(c + 1) * RPC],
            "xb": xb,
            "xsb": xsb,
            "csw": csw,
            "i2": i2,
            "tidx": tidx,
        })
    return maps


def kernel(**inputs):
    global last_results
    from concourse.bass_utils import run_bass_kernel_spmd

    A = np.ascontiguousarray(np.asarray(inputs["A"], dtype=np.float32))
    X = np.ascontiguousarray(np.asarray(inputs["X"], dtype=np.float32))
    alpha = np.float32(np.asarray(inputs["alpha"]))
    k = int(np.asarray(inputs["k"]))
    assert A.shape == (N, N) and X.shape == (N, D)
    if k != K or alpha == 0.0:
        # Safety net for an unexpected k (or alpha=0): exact host computation.
        idx = np.argsort(-A, axis=1, kind="stable")[:, :k]
        agg = X[idx].sum(axis=1, dtype=np.float32)
        return (X + alpha * agg).astype(np.float32)

    nc = _build_cached()
    in_maps = make_in_maps(A, X, alpha)

    trace = bool(int(os.environ.get("TOPK_TRACE", "0")))
    res = run_bass_kernel_spmd(nc, in_maps, core_ids=list(range(NCORES)),
                               trace=trace)
    last_results = res

    # per-core output is [D, RPC] bf16; un-transpose and stack rows
    out = np.concatenate(
        [r["out"].astype(np.float32).T for r in res.results], axis=0)
    accs = np.concatenate([r["count"] for r in res.results], axis=0)[:, 0]

    # Host fallback for rows where the device selection is not exactly top-k
    # (fp16 boundary ties, prefilter mate collisions, segment overflow,
    # Sign hitting exact zero).
    bad = np.flatnonzero(accs != np.float32(2 * K - N))
    for r in bad:
        order = np.argsort(-A[r], kind="stable")[:K]
        out[r] = X[r] + alpha * X[order].sum(axis=0, dtype=np.float32)

    return np.ascontiguousarray(out, dtype=np.float32)
